# revision 2
# baseline (speedup 1.0000x reference)
"""MoE layer (dense top-2 routing) on 8 Trainium2 NeuronCores.

The reference computes all 8 experts densely, then combines with top-2
softmax gate scores -- 6 of 8 expert outputs get exactly zero weight.
This kernel exploits that: routing (gate logits -> top-2 -> softmax) runs
on host in fp32 (the 2nd/3rd logit gap is >>> fp32 rounding noise, so
selection matches the reference bit-for-bit), and the device computes
only the selected expert matmuls.

Sharding: expert-parallel. Core e owns expert e's weight matrix [D,O]
(resident in SBUF) and processes the tokens routed to expert e (gathered
and transposed on host), padded to a static capacity CAP = NT*128 where
NT = ceil(max_e count_e / 128). Device computes Y = score * (X_e @ W_e)
in fp32r (full PE rate). Host scatter-adds the two scaled expert rows
per token and the gated bias term.

Per-core kernel layout:
  xT  [D=1024, CAP]   gathered token slice, transposed (host-prepped)
  w   [D=1024, O=1024] this expert's weights
  sc  [128, NT]        per-row gate score, sc[p,t] = score of row t*128+p
  out [CAP, O=1024]    scaled expert output rows
"""

import numpy as np

B, S, D, O, E = 4, 2048, 1024, 1024, 8
NCORES = 8
P = 128
KT = D // P          # k tiles over D
OS = 512             # psum moving width (>=256 keeps fp32r at full rate)
OT = O // OS
XCHUNK = 256         # token columns per x DMA tile (2 token-tiles)


def build_nc(nt=18, reps=1):
    import concourse.bacc as bacc
    import concourse.mybir as mybir
    import concourse.tile as tile

    f32 = mybir.dt.float32
    f32r = mybir.dt.float32r

    cap = nt * P
    nchunk = cap // XCHUNK

    nc = bacc.Bacc()
    xT_d = nc.declare_dram_parameter("xT", [D, cap], f32, isOutput=False)
    w_d = nc.declare_dram_parameter("w", [D, O], f32r, isOutput=False)
    sc_d = nc.declare_dram_parameter("sc", [P, nt], f32, isOutput=False)
    out_d = nc.declare_dram_parameter("out", [cap, O], f32, isOutput=True)

    with tile.TileContext(nc) as tc:
        with (
            tc.tile_pool(name="wp", bufs=32) as w_pool,
            tc.tile_pool(name="xp", bufs=24) as x_pool,
            tc.tile_pool(name="scp", bufs=2) as sc_pool,
            tc.tile_pool(name="accp", bufs=4) as acc_pool,
            tc.tile_pool(name="ps", bufs=8, space="PSUM") as ps_pool,
        ):
            def one_rep():
                sc_t = sc_pool.tile([P, nt], f32, tag="sc", name="sc")
                nc.sync.dma_start(out=sc_t[:], in_=sc_d[:])
                wt = {}
                for ot in range(OT):
                    osl = slice(ot * OS, (ot + 1) * OS)
                    for k in range(KT):
                        t = w_pool.tile([P, OS], f32r, tag="w",
                                        name=f"w{k}_{ot}")
                        nc.sync.dma_start(
                            out=t[:], in_=w_d[k * P:(k + 1) * P, osl])
                        wt[(k, ot)] = t
                xt = {}
                for c in range(nchunk):
                    csl = slice(c * XCHUNK, (c + 1) * XCHUNK)
                    for k in range(KT):
                        t = x_pool.tile([P, XCHUNK], f32r, tag="x",
                                        name=f"x{c}_{k}")
                        nc.sync.dma_start(
                            out=t[:],
                            in_=xT_d[k * P:(k + 1) * P, csl].bitcast(f32r))
                        xt[(c, k)] = t
                for tt in range(nt):
                    c, col = tt // 2, (tt % 2) * P
                    tsl = slice(tt * P, (tt + 1) * P)
                    for ot in range(OT):
                        osl = slice(ot * OS, (ot + 1) * OS)
                        ps = ps_pool.tile([P, OS], f32, tag="ps", name="ps")
                        for k in range(KT):
                            nc.tensor.matmul(
                                ps[:],
                                lhsT=xt[(c, k)][:, col:col + P],
                                rhs=wt[(k, ot)][:],
                                start=(k == 0), stop=(k == KT - 1))
                        acc = acc_pool.tile([P, OS], f32, tag="acc",
                                            name="acc")
                        nc.scalar.mul(acc[:], ps[:], mul=sc_t[:, tt:tt + 1])
                        nc.sync.dma_start(out=out_d[tsl, osl], in_=acc[:])

            for _rep in range(reps):
                one_rep()

    nc.compile()
    return nc


_cache = {}


def _get_nc(nt):
    if nt not in _cache:
        _cache[nt] = build_nc(nt=nt)
    return _cache[nt]


def route(x, gate_w, gate_b):
    """Host top-2 routing in fp32: returns (e1, e2, p1, p2) per token."""
    x2 = np.asarray(x, np.float32).reshape(B * S, D)
    logits = x2 @ np.asarray(gate_w, np.float32).T + np.asarray(
        gate_b, np.float32)
    e1 = logits.argmax(-1)
    rows = np.arange(len(e1))
    l1 = logits[rows, e1]
    masked = logits.copy()
    masked[rows, e1] = -np.inf
    e2 = masked.argmax(-1)
    l2 = masked[rows, e2]
    # softmax over the kept pair
    p2 = 1.0 / (1.0 + np.exp(l1 - l2))
    p1 = 1.0 - p2
    return x2, e1, e2, p1, p2


def make_in_maps(x, gate_w, gate_b, expert_w, expert_b):
    x2, e1, e2, p1, p2 = route(x, gate_w, gate_b)
    idxs, scores = [], []
    for e in range(E):
        i1 = np.nonzero(e1 == e)[0]
        i2 = np.nonzero(e2 == e)[0]
        idxs.append(np.concatenate([i1, i2]))
        scores.append(np.concatenate([p1[i1], p2[i2]]).astype(np.float32))
    nt = max(-(-max(len(i) for i in idxs) // P), 1)
    cap = nt * P
    in_maps = []
    for e in range(E):
        n = len(idxs[e])
        xT = np.zeros((D, cap), np.float32)
        xT[:, :n] = x2[idxs[e]].T
        sc = np.zeros(cap, np.float32)
        sc[:n] = scores[e]
        sc = np.ascontiguousarray(sc.reshape(nt, P).T)
        w = np.ascontiguousarray(np.asarray(expert_w[e], np.float32))
        in_maps.append({"xT": xT, "w": w, "sc": sc})
    return in_maps, idxs, nt, (e1, e2, p1, p2)


def kernel(x, gate_w, gate_b, expert_w, expert_b):
    from concourse.bass_utils import run_bass_kernel_spmd

    in_maps, idxs, nt, (e1, e2, p1, p2) = make_in_maps(
        x, gate_w, gate_b, expert_w, expert_b)
    nc = _get_nc(nt)
    res = run_bass_kernel_spmd(nc, in_maps, list(range(NCORES)))
    out = np.zeros((B * S, O), np.float32)
    for e in range(E):
        n = len(idxs[e])
        out[idxs[e]] += res.results[e]["out"][:n]
    eb = np.asarray(expert_b, np.float32)
    out += p1[:, None].astype(np.float32) * eb[e1]
    out += p2[:, None].astype(np.float32) * eb[e2]
    return out.reshape(B, S, O)


# revision 3
# speedup vs baseline: 41.5326x; 41.5326x over previous
"""MoE layer (dense top-2 routing) on 8 Trainium2 NeuronCores.

The reference computes all 8 experts densely, then combines with top-2
softmax gate scores -- 6 of 8 expert outputs get exactly zero weight.
This kernel exploits that: routing (gate logits -> top-2 -> softmax) runs
on host in fp32 (the 2nd/3rd logit gap is >> fp32 rounding noise, so
selection matches the reference exactly), and the device computes only
the selected expert matmuls.

Sharding: expert-parallel. Core e owns expert e's weight matrix [D,O]
and processes the tokens routed to expert e (gathered on host), padded
to a static capacity CAP = NT*128 where NT = ceil(max_e count_e / 128).
Device computes Y = score * (X_e @ W_e) in fp32r (full PE rate). Host
scatter-adds the two scaled expert rows per token plus the gated bias.

DMA layout: x and w are pre-tiled on host into the exact SBUF layout so
every dma_start moves >=1MB of per-partition-contiguous data (small
strided DMAs are descriptor-dominated on TRN2):
  xg [nchunk, 128, KT*XCHUNK]  xg[c, p, k*XCHUNK+j] = x[tok c*XCHUNK+j, k*128+p]
  wg [OT, 128, KT*OS]          wg[ot, p, k*OS+j]    = w[k*128+p, ot*OS+j]
  sc [128, NT]                 sc[p, t] = gate score of row t*128+p
  out [CAP, O]
"""

import numpy as np

B, S, D, O, E = 4, 2048, 1024, 1024, 8
NCORES = 8
P = 128
KT = D // P          # k tiles over D
OS = 512             # psum moving width (>=256 keeps fp32r at full rate)
OT = O // OS
XCHUNK = 256         # tokens per x DMA block (2 token-tiles, 1MB)


def build_nc(nt=18, reps=1):
    import concourse.bacc as bacc
    import concourse.mybir as mybir
    import concourse.tile as tile

    f32 = mybir.dt.float32
    f32r = mybir.dt.float32r

    cap = nt * P
    assert cap % XCHUNK == 0
    nchunk = cap // XCHUNK

    nc = bacc.Bacc()
    xg_d = nc.declare_dram_parameter("xg", [nchunk, P, KT * XCHUNK], f32,
                                     isOutput=False)
    wg_d = nc.declare_dram_parameter("wg", [OT, P, KT * OS], f32r,
                                     isOutput=False)
    sc_d = nc.declare_dram_parameter("sc", [P, nt], f32, isOutput=False)
    out_d = nc.declare_dram_parameter("out", [cap, O], f32, isOutput=True)

    with tile.TileContext(nc) as tc:
        with (
            tc.tile_pool(name="wp", bufs=4) as w_pool,
            tc.tile_pool(name="xp", bufs=4) as x_pool,
            tc.tile_pool(name="scp", bufs=2) as sc_pool,
            tc.tile_pool(name="accp", bufs=4) as acc_pool,
            tc.tile_pool(name="ps", bufs=8, space="PSUM") as ps_pool,
        ):
            def one_rep():
                sc_t = sc_pool.tile([P, nt], f32, tag="sc", name="sc")
                nc.sync.dma_start(out=sc_t[:], in_=sc_d[:])
                wt = []
                for ot in range(OT):
                    t = w_pool.tile([P, KT * OS], f32r, tag="w",
                                    name=f"w{ot}")
                    nc.sync.dma_start(out=t[:], in_=wg_d[ot])
                    wt.append(t)
                xt = []
                for c in range(nchunk):
                    t = x_pool.tile([P, KT * XCHUNK], f32r, tag="x",
                                    name=f"x{c}")
                    nc.sync.dma_start(out=t[:], in_=xg_d[c].bitcast(f32r))
                    xt.append(t)
                for tt in range(nt):
                    c, col = tt // 2, (tt % 2) * P
                    tsl = slice(tt * P, (tt + 1) * P)
                    acc = acc_pool.tile([P, O], f32, tag="acc", name="acc")
                    for ot in range(OT):
                        ps = ps_pool.tile([P, OS], f32, tag="ps", name="ps")
                        for k in range(KT):
                            nc.tensor.matmul(
                                ps[:],
                                lhsT=xt[c][:, k * XCHUNK + col:
                                           k * XCHUNK + col + P],
                                rhs=wt[ot][:, k * OS:(k + 1) * OS],
                                start=(k == 0), stop=(k == KT - 1))
                        nc.scalar.mul(acc[:, ot * OS:(ot + 1) * OS], ps[:],
                                      mul=sc_t[:, tt:tt + 1])
                    nc.sync.dma_start(out=out_d[tsl, :], in_=acc[:])

            for _rep in range(reps):
                one_rep()

    nc.compile()
    return nc


_cache = {}


def _get_nc(nt):
    if nt not in _cache:
        _cache[nt] = build_nc(nt=nt)
    return _cache[nt]


def route(x, gate_w, gate_b):
    """Host top-2 routing in fp32: returns (e1, e2, p1, p2) per token."""
    x2 = np.asarray(x, np.float32).reshape(B * S, D)
    logits = x2 @ np.asarray(gate_w, np.float32).T + np.asarray(
        gate_b, np.float32)
    e1 = logits.argmax(-1)
    rows = np.arange(len(e1))
    l1 = logits[rows, e1]
    masked = logits.copy()
    masked[rows, e1] = -np.inf
    e2 = masked.argmax(-1)
    l2 = masked[rows, e2]
    # softmax over the kept pair
    p2 = 1.0 / (1.0 + np.exp(l1 - l2))
    p1 = 1.0 - p2
    return x2, e1, e2, p1, p2


def make_in_maps(x, gate_w, gate_b, expert_w, expert_b):
    x2, e1, e2, p1, p2 = route(x, gate_w, gate_b)
    idxs, scores = [], []
    for e in range(E):
        i1 = np.nonzero(e1 == e)[0]
        i2 = np.nonzero(e2 == e)[0]
        idxs.append(np.concatenate([i1, i2]))
        scores.append(np.concatenate([p1[i1], p2[i2]]).astype(np.float32))
    nt = max(-(-max(len(i) for i in idxs) // P), 1)
    if nt % 2:
        nt += 1  # keep cap divisible by XCHUNK (2 tiles per chunk)
    cap = nt * P
    nchunk = cap // XCHUNK
    in_maps = []
    for e in range(E):
        n = len(idxs[e])
        xx = np.zeros((cap, D), np.float32)
        xx[:n] = x2[idxs[e]]
        # [cap, D] -> [nchunk, XCHUNK, KT, P] -> [nchunk, P, KT, XCHUNK]
        xg = np.ascontiguousarray(
            xx.reshape(nchunk, XCHUNK, KT, P).transpose(0, 3, 2, 1)
        ).reshape(nchunk, P, KT * XCHUNK)
        sc = np.zeros(cap, np.float32)
        sc[:n] = scores[e]
        sc = np.ascontiguousarray(sc.reshape(nt, P).T)
        w = np.asarray(expert_w[e], np.float32)
        wg = np.ascontiguousarray(
            w.reshape(KT, P, OT, OS).transpose(2, 1, 0, 3)
        ).reshape(OT, P, KT * OS)
        in_maps.append({"xg": xg, "wg": wg, "sc": sc})
    return in_maps, idxs, nt, (e1, e2, p1, p2)


def kernel(x, gate_w, gate_b, expert_w, expert_b):
    from concourse.bass_utils import run_bass_kernel_spmd

    in_maps, idxs, nt, (e1, e2, p1, p2) = make_in_maps(
        x, gate_w, gate_b, expert_w, expert_b)
    nc = _get_nc(nt)
    res = run_bass_kernel_spmd(nc, in_maps, list(range(NCORES)))
    out = np.zeros((B * S, O), np.float32)
    for e in range(E):
        n = len(idxs[e])
        out[idxs[e]] += res.results[e]["out"][:n]
    eb = np.asarray(expert_b, np.float32)
    out += p1[:, None].astype(np.float32) * eb[e1]
    out += p2[:, None].astype(np.float32) * eb[e2]
    return out.reshape(B, S, O)
